# revision 4
# baseline (speedup 1.0000x reference)
"""AssociationLoss kernel for Trainium2, distributed over 8 NeuronCores.

Math (reference): BCE-with-logits over the [P, C] cosine-similarity matrix
between prev_feat (detached) and cur_feat, with labels = (prev_ids == cur_ids).

Per element: loss = softplus(x) - x * y  where y in {0, 1}
           = softplus(s * x) with s = 1 - 2*y  (sign flip on matching pairs)

Distribution: row-parallel on P.  Each core holds a [P/8, D] prev shard plus
the full cur side, computes its [P/8, C] tile's loss-sum on device, and the
host sums the 8 partial scalars and divides by P*C (the "unshard" step).

Device pipeline per core:
  - normalize cur features (Square+accum -> sqrt -> reciprocal, then scale the
    pre-transposed cur features to bf16 "chatT")
  - prev norms stay as per-partition scalars (folded into the Exp activation
    scale), so prev features go to the matmul raw (bf16 "pfT")
  - TensorE: x_raw[p, c] = pfT.T @ chatT  (PSUM, fp32, K=256 accumulation)
  - VectorE: t' = (cid == pid) - 0.5   (bf16, fused tensor_scalar)
             v  = x_raw * t'
  - ScalarE: e = Exp(v * (-2 * invnp_p)); partial = sum Ln(1 + e)  (accum_out)
  - cross-partition sum via ones-matmul, DMA scalar out
"""

import numpy as np
import ml_dtypes

import concourse.bass as bass
import concourse.tile as tile
import concourse.mybir as mybir
from concourse import bacc
from concourse.bass_utils import run_bass_kernel_spmd

F32 = mybir.dt.float32
BF16 = mybir.dt.bfloat16
AF = mybir.ActivationFunctionType
OP = mybir.AluOpType

P, C, D = 8192, 8192, 256
NCORES = 8
PS = P // NCORES          # 1024 prev rows per core
NPJ = PS // 128           # 8 prev chunks of 128 partitions
NCH = C // 128            # 64 cur chunks
CG = 2048                 # c-group width processed per PSUM tile
NCG = C // CG             # 4 c-groups
ND = D // 128             # 2 contraction chunks


def _build():
    nc = bacc.Bacc(None, target_bir_lowering=False, debug=False, num_devices=NCORES)

    pf_n = nc.dram_tensor("pf_n", [128, NPJ, D], F32, kind="ExternalInput").ap()
    pfT = nc.dram_tensor("pfT", [128, ND, PS], F32, kind="ExternalInput").ap()
    cf_n = nc.dram_tensor("cf_n", [128, NCH, D], F32, kind="ExternalInput").ap()
    cfT = nc.dram_tensor("cfT", [128, ND, C], F32, kind="ExternalInput").ap()
    cid_b = nc.dram_tensor("cid_b", [128, C], BF16, kind="ExternalInput").ap()
    pid = nc.dram_tensor("pid", [128, NPJ], F32, kind="ExternalInput").ap()
    out = nc.dram_tensor("out", [1, 1], F32, kind="ExternalOutput").ap()

    scr = nc.dram_tensor("nscr", [C], F32).ap()  # invnc roundtrip scratch

    with tile.TileContext(nc) as tc:
        with (
            tc.tile_pool(name="singles", bufs=1) as singles,
            tc.tile_pool(name="cfchunk", bufs=4) as cfchunk,
            tc.tile_pool(name="cftchunk", bufs=2) as cftchunk,
            tc.tile_pool(name="sqd", bufs=2) as sqdp,
            tc.tile_pool(name="psum", bufs=2, space="PSUM") as psum,
            tc.tile_pool(name="work", bufs=3) as work,
        ):
            # ---- constants / persistent tiles ----
            pfT_bf = singles.tile([128, ND, PS], BF16)
            chatT = singles.tile([128, ND, C], BF16)
            cid_sb = singles.tile([128, C], BF16)
            pid_sb = singles.tile([128, NPJ], F32)
            ssqp = singles.tile([128, NPJ], F32)
            invnp = singles.tile([128, NPJ], F32)
            m2invnp = singles.tile([128, NPJ], F32)
            ssqc = singles.tile([128, NCH], F32)
            invnc = singles.tile([128, NCH], F32)
            invnc_bc = singles.tile([128, C], F32)
            acc = singles.tile([128, NPJ * NCG], F32)
            ones = singles.tile([128, 1], F32)
            nc.vector.memset(ones[:], 1.0)

            # ---- prev side ----
            pf_sb = singles.tile([128, NPJ, D], F32)
            nc.sync.dma_start(pf_sb[:], pf_n)
            for j in range(NPJ):
                sqd = sqdp.tile([128, D], BF16)
                nc.scalar.activation(sqd[:], pf_sb[:, j], AF.Square,
                                     accum_out=ssqp[:, j : j + 1])
            nc.scalar.activation(invnp[:], ssqp[:], AF.Sqrt)  # = norm, inverted below
            nc.vector.reciprocal(invnp[:], invnp[:])
            nc.vector.tensor_scalar_mul(m2invnp[:], invnp[:], -2.0)

            pfT_f = singles.tile([128, ND, PS], F32)
            nc.sync.dma_start(pfT_f[:], pfT)
            nc.vector.tensor_copy(pfT_bf[:], pfT_f[:])

            nc.sync.dma_start(pid_sb[:], pid)
            nc.sync.dma_start(cid_sb[:], cid_b)

            # ---- cur norms (streamed chunks) ----
            for ch in range(NCH):
                cfch = cfchunk.tile([128, D], F32)
                nc.sync.dma_start(cfch[:], cf_n[:, ch])
                sqd = sqdp.tile([128, D], BF16)
                nc.scalar.activation(sqd[:], cfch[:], AF.Square,
                                     accum_out=ssqc[:, ch : ch + 1])
            nc.scalar.activation(invnc[:], ssqc[:], AF.Sqrt)
            nc.vector.reciprocal(invnc[:], invnc[:])

            # roundtrip through DRAM to change layout [128, 64] -> [1, 8192],
            # then broadcast-read into all 128 partitions
            nc.sync.dma_start(scr.rearrange("(ch p) -> p ch", p=128), invnc[:])
            nc.sync.dma_start(invnc_bc[:], scr[None, :].broadcast_to((128, C)))

            # ---- scale pre-transposed cur features by 1/||c|| -> bf16 chatT ----
            H = C // 2
            for dc in range(ND):
                for h in range(2):
                    cft = cftchunk.tile([128, H], F32)
                    nc.sync.dma_start(cft[:], cfT[:, dc, h * H : (h + 1) * H])
                    nc.vector.tensor_tensor(
                        out=chatT[:, dc, h * H : (h + 1) * H],
                        in0=cft[:],
                        in1=invnc_bc[:, h * H : (h + 1) * H],
                        op=OP.mult,
                    )

            # ---- main loop ----
            for j in range(NPJ):
                for cg in range(NCG):
                    ps = psum.tile([128, CG], F32)
                    for cs in range(CG // 512):
                        c0 = cg * CG + cs * 512
                        for dc in range(ND):
                            nc.tensor.matmul(
                                ps[:, cs * 512 : (cs + 1) * 512],
                                pfT_bf[:, dc, j * 128 : (j + 1) * 128],
                                chatT[:, dc, c0 : c0 + 512],
                                start=(dc == 0),
                                stop=(dc == ND - 1),
                            )
                    tp = work.tile([128, CG], BF16, tag="tp")
                    nc.vector.tensor_scalar(
                        out=tp[:],
                        in0=cid_sb[:, cg * CG : (cg + 1) * CG],
                        scalar1=pid_sb[:, j : j + 1],
                        scalar2=-0.5,
                        op0=OP.is_equal,
                        op1=OP.add,
                    )
                    v = work.tile([128, CG], BF16, tag="v")
                    nc.vector.tensor_tensor(out=v[:], in0=ps[:], in1=tp[:], op=OP.mult)
                    e = work.tile([128, CG], BF16, tag="e")
                    nc.scalar.activation(e[:], v[:], AF.Exp,
                                         scale=m2invnp[:, j : j + 1])
                    ln = work.tile([128, CG], BF16, tag="ln")
                    nc.scalar.activation(ln[:], e[:], AF.Ln, bias=1.0,
                                         accum_out=acc[:, j * NCG + cg : j * NCG + cg + 1])

            # ---- reduction to a single scalar ----
            tot = singles.tile([128, 1], F32)
            nc.vector.tensor_reduce(tot[:], acc[:], axis=mybir.AxisListType.X,
                                    op=OP.add)
            ps1 = psum.tile([1, 1], F32, tag="ps")
            nc.tensor.matmul(ps1[:], tot[:], ones[:], start=True, stop=True)
            res = singles.tile([1, 1], F32)
            nc.vector.tensor_copy(res[:], ps1[:])
            nc.sync.dma_start(out, res[:])

    nc.compile()
    return nc


_NC_CACHE = None


def _get_nc():
    global _NC_CACHE
    if _NC_CACHE is None:
        _NC_CACHE = _build()
    return _NC_CACHE


def _encode_ids_u16(ids: np.ndarray) -> np.ndarray:
    # Map id -> bf16 bit-pattern id+128: all distinct, normal (no denormal/NaN)
    return (np.asarray(ids).astype(np.int64) + 128).astype(np.uint16)


def make_in_maps(prev_feat, cur_feat, prev_ids, cur_ids):
    prev_feat = np.asarray(prev_feat, dtype=np.float32)
    cur_feat = np.asarray(cur_feat, dtype=np.float32)
    prev_ids = np.asarray(prev_ids)
    cur_ids = np.asarray(cur_ids)

    cid_bf = _encode_ids_u16(cur_ids).view(ml_dtypes.bfloat16)  # [C]
    cid_b = np.ascontiguousarray(np.broadcast_to(cid_bf[None, :], (128, C)))

    cf_n = np.ascontiguousarray(cur_feat.reshape(NCH, 128, D).transpose(1, 0, 2))
    cfT = np.ascontiguousarray(
        cur_feat.T.reshape(ND, 128, C).transpose(1, 0, 2)
    )

    in_maps = []
    for k in range(NCORES):
        sl = slice(k * PS, (k + 1) * PS)
        pf = prev_feat[sl]  # [PS, D]
        pf_n = np.ascontiguousarray(pf.reshape(NPJ, 128, D).transpose(1, 0, 2))
        pfT = np.ascontiguousarray(pf.T.reshape(ND, 128, PS).transpose(1, 0, 2))
        pid_enc = (_encode_ids_u16(prev_ids[sl]).astype(np.uint32) << 16).view(
            np.float32
        )
        pid = np.ascontiguousarray(pid_enc.reshape(NPJ, 128).T)  # [128, NPJ]
        in_maps.append(
            dict(pf_n=pf_n, pfT=pfT, cf_n=cf_n, cfT=cfT, cid_b=cid_b, pid=pid)
        )
    return in_maps


def run(prev_feat, cur_feat, prev_ids, cur_ids, trace=False, **kw):
    nc = _get_nc()
    in_maps = make_in_maps(prev_feat, cur_feat, prev_ids, cur_ids)
    res = run_bass_kernel_spmd(nc, in_maps, core_ids=list(range(NCORES)),
                               trace=trace, **kw)
    partials = np.array(
        [res.results[i]["out"][0, 0] for i in range(NCORES)], dtype=np.float64
    )
    loss = (partials.sum() / (P * C)).astype(np.float32)
    return np.float32(loss), res


def kernel(prev_feat, cur_feat, prev_ids, cur_ids):
    loss, _ = run(prev_feat, cur_feat, prev_ids, cur_ids, trace=False)
    return np.asarray(loss, dtype=np.float32)


# revision 5
# speedup vs baseline: 1.5900x; 1.5900x over previous
"""AssociationLoss kernel for Trainium2, distributed over 8 NeuronCores.

Math (reference): BCE-with-logits over the [P, C] cosine-similarity matrix
between prev_feat (detached) and cur_feat, with labels = (prev_ids == cur_ids).

Per element: loss = softplus(x) - x * y  where y in {0, 1}
           = softplus(s * x) with s = 1 - 2*y  (sign flip on matching pairs)

softplus on [-1, 1] (cosine bound) via a single LUT pass:
    softplus(z) = silu(B*z)/B + ln2 + C0  +/- 4e-4   (B = 0.490068)
(odd parts match exactly since silu(t)/t' has slope 1/2 at 0; the residual is
even and within 4e-4 of the constant C0 over the full domain.)

Distribution: row-parallel on P.  Each core holds a [P/8, D] prev shard plus a
[C/8, D] cur shard; cur shards are normalized locally and all-gathered (bf16,
transposed layout) so every core can compute its [P/8, C] tile.  The host sums
the 8 partial scalars and applies the constant offset (the "unshard" step).

Device pipeline per core:
  - Square+accum row norms; 1/sqrt via Exp(-0.5*Ln(ssq)) (same LUT table set)
  - cur shard: scale transposed features by 1/||c|| -> bf16 chatT shard
  - AllGather chatT shards -> full chatT [D, C]
  - TensorE: x_raw[p, c] = pfT_raw.T @ chatT  (PSUM fp32, K=256, prev norm
    deferred to the activation scale)
  - VectorE: t' = (cid == pid) - 0.5  (bf16); v = x_raw * t'   [v = -s*x/2]
  - ScalarE: partial += sum silu((-2*B*invnp_p) * v)  (accum_out)
  - cross-partition sum via ones-matmul, DMA scalar out
"""

import numpy as np
import ml_dtypes

import concourse.bass as bass
import concourse.tile as tile
import concourse.mybir as mybir
from concourse import bacc
from concourse.bass_utils import run_bass_kernel_spmd

F32 = mybir.dt.float32
BF16 = mybir.dt.bfloat16
AF = mybir.ActivationFunctionType
OP = mybir.AluOpType

P, C, D = 8192, 8192, 256
NCORES = 8
PS = P // NCORES          # 1024 prev rows per core
CS = C // NCORES          # 1024 cur rows per core
NPJ = PS // 128           # 8 prev chunks
NCJ = CS // 128           # 8 cur chunks per shard
CG = 2048                 # c-group width per PSUM tile
NCG = C // CG
ND = D // 128             # 2 contraction chunks

SILU_B = 0.490068
SILU_C0 = 0.00039011
LN2 = float(np.log(2.0))


def _build(mode="silu"):
    nc = bacc.Bacc(None, target_bir_lowering=False, debug=False, num_devices=NCORES)

    pf_n = nc.dram_tensor("pf_n", [128, NPJ, D], F32, kind="ExternalInput").ap()
    pfT = nc.dram_tensor("pfT", [128, ND, PS], F32, kind="ExternalInput").ap()
    cfs_n = nc.dram_tensor("cfs_n", [128, NCJ, D], F32, kind="ExternalInput").ap()
    cfsT = nc.dram_tensor("cfsT", [128, ND, CS], F32, kind="ExternalInput").ap()
    cid_b = nc.dram_tensor("cid_b", [128, C], BF16, kind="ExternalInput").ap()
    pid = nc.dram_tensor("pid", [128, NPJ], F32, kind="ExternalInput").ap()
    out = nc.dram_tensor("out", [1, 1], F32, kind="ExternalOutput").ap()

    scr = nc.dram_tensor("nscr", [CS], F32).ap()  # invnc roundtrip scratch
    gin = nc.dram_tensor("gin", [ND * 128, CS], BF16).ap()
    gout = nc.dram_tensor("gout", [NCORES * ND * 128, CS], BF16,
                          addr_space="Shared").ap()

    with tile.TileContext(nc) as tc:
        with (
            tc.tile_pool(name="singles", bufs=1) as singles,
            tc.tile_pool(name="sqd", bufs=2) as sqdp,
            tc.tile_pool(name="psum", bufs=2, space="PSUM") as psum,
            tc.tile_pool(name="work", bufs=3) as work,
        ):
            # ---- persistent tiles ----
            pfT_bf = singles.tile([128, ND, PS], BF16)
            chatT = singles.tile([128, ND, C], BF16)
            cid_sb = singles.tile([128, C], BF16)
            pid_sb = singles.tile([128, NPJ], F32)
            ssqp = singles.tile([128, NPJ], F32)
            invnp = singles.tile([128, NPJ], F32)
            snp = singles.tile([128, NPJ], F32)      # activation scale per chunk
            ssqc = singles.tile([128, NCJ], F32)
            invnc = singles.tile([128, NCJ], F32)
            invnc_bc = singles.tile([128, CS], F32)
            chatTs = singles.tile([128, ND, CS], BF16)
            acc = singles.tile([128, NPJ * NCG], F32)
            ones = singles.tile([128, 1], F32)
            nc.vector.memset(ones[:], 1.0)

            # ---- DMAs in ----
            pf_sb = singles.tile([128, NPJ, D], F32)
            nc.sync.dma_start(pf_sb[:], pf_n)
            cf_sb = singles.tile([128, NCJ, D], F32)
            nc.sync.dma_start(cf_sb[:], cfs_n)
            pfT_f = singles.tile([128, ND, PS], F32)
            nc.sync.dma_start(pfT_f[:], pfT)
            cfsT_f = singles.tile([128, ND, CS], F32)
            nc.sync.dma_start(cfsT_f[:], cfsT)
            nc.sync.dma_start(pid_sb[:], pid)
            nc.sync.dma_start(cid_sb[:], cid_b)

            nc.vector.tensor_copy(pfT_bf[:], pfT_f[:])

            # ---- norms: ssq via Square+accum; 1/sqrt via Exp(-0.5*Ln) ----
            for j in range(NPJ):
                sqd = sqdp.tile([128, D], BF16)
                nc.scalar.activation(sqd[:], pf_sb[:, j], AF.Square,
                                     accum_out=ssqp[:, j : j + 1])
            for j in range(NCJ):
                sqd = sqdp.tile([128, D], BF16)
                nc.scalar.activation(sqd[:], cf_sb[:, j], AF.Square,
                                     accum_out=ssqc[:, j : j + 1])
            nc.scalar.activation(invnp[:], ssqp[:], AF.Ln)
            nc.scalar.activation(invnp[:], invnp[:], AF.Exp, scale=-0.5)
            nc.scalar.activation(invnc[:], ssqc[:], AF.Ln)
            nc.scalar.activation(invnc[:], invnc[:], AF.Exp, scale=-0.5)
            scale_const = -2.0 * (SILU_B if mode == "silu" else 1.0)
            nc.vector.tensor_scalar_mul(snp[:], invnp[:], scale_const)

            # ---- normalize cur shard (transposed layout) -> bf16 ----
            nc.sync.dma_start(scr.rearrange("(ch p) -> p ch", p=128), invnc[:])
            nc.sync.dma_start(invnc_bc[:], scr[None, :].broadcast_to((128, CS)))
            for dc in range(ND):
                nc.vector.tensor_tensor(out=chatTs[:, dc], in0=cfsT_f[:, dc],
                                        in1=invnc_bc[:], op=OP.mult)

            # ---- AllGather cur shards ----
            nc.sync.dma_start(gin.rearrange("(dc p) c -> p dc c", p=128), chatTs[:])
            nc.gpsimd.collective_compute(
                "AllGather",
                OP.bypass,
                replica_groups=[list(range(NCORES))],
                ins=[gin],
                outs=[gout],
            )
            gv = gout.rearrange("(s dc p) c -> p dc s c", p=128, dc=ND)
            for s in range(NCORES):
                for dc in range(ND):
                    nc.sync.dma_start(chatT[:, dc, s * CS : (s + 1) * CS],
                                      gv[:, dc, s])

            # ---- main loop ----
            for j in range(NPJ):
                for cg in range(NCG):
                    ps = psum.tile([128, CG], F32)
                    for cs in range(CG // 512):
                        c0 = cg * CG + cs * 512
                        for dc in range(ND):
                            nc.tensor.matmul(
                                ps[:, cs * 512 : (cs + 1) * 512],
                                pfT_bf[:, dc, j * 128 : (j + 1) * 128],
                                chatT[:, dc, c0 : c0 + 512],
                                start=(dc == 0),
                                stop=(dc == ND - 1),
                            )
                    tp = work.tile([128, CG], BF16, tag="tp")
                    nc.vector.tensor_scalar(
                        out=tp[:],
                        in0=cid_sb[:, cg * CG : (cg + 1) * CG],
                        scalar1=pid_sb[:, j : j + 1],
                        scalar2=-0.5,
                        op0=OP.is_equal,
                        op1=OP.add,
                    )
                    v = work.tile([128, CG], BF16, tag="v")
                    nc.vector.tensor_tensor(out=v[:], in0=ps[:], in1=tp[:], op=OP.mult)
                    aidx = acc[:, j * NCG + cg : j * NCG + cg + 1]
                    if mode == "silu":
                        sdummy = work.tile([128, CG], BF16, tag="sdummy")
                        nc.scalar.activation(sdummy[:], v[:], AF.Silu,
                                             scale=snp[:, j : j + 1],
                                             accum_out=aidx)
                    else:
                        e = work.tile([128, CG], BF16, tag="e")
                        nc.scalar.activation(e[:], v[:], AF.Exp,
                                             scale=snp[:, j : j + 1])
                        ln = work.tile([128, CG], BF16, tag="ln")
                        nc.scalar.activation(ln[:], e[:], AF.Ln, bias=1.0,
                                             accum_out=aidx)

            # ---- reduce to one scalar ----
            tot = singles.tile([128, 1], F32)
            nc.vector.tensor_reduce(tot[:], acc[:], axis=mybir.AxisListType.X,
                                    op=OP.add)
            ps1 = psum.tile([1, 1], F32, tag="ps")
            nc.tensor.matmul(ps1[:], tot[:], ones[:], start=True, stop=True)
            res = singles.tile([1, 1], F32)
            nc.vector.tensor_copy(res[:], ps1[:])
            nc.sync.dma_start(out, res[:])

    nc.compile()
    return nc


_NC_CACHE = {}


def _get_nc(mode="silu"):
    if mode not in _NC_CACHE:
        _NC_CACHE[mode] = _build(mode)
    return _NC_CACHE[mode]


def _encode_ids_u16(ids):
    # id -> bf16 bit-pattern id+128: all distinct, normal (no denormal/NaN)
    return (np.asarray(ids).astype(np.int64) + 128).astype(np.uint16)


def make_in_maps(prev_feat, cur_feat, prev_ids, cur_ids):
    prev_feat = np.asarray(prev_feat, dtype=np.float32)
    cur_feat = np.asarray(cur_feat, dtype=np.float32)
    prev_ids = np.asarray(prev_ids)
    cur_ids = np.asarray(cur_ids)

    cid_bf = _encode_ids_u16(cur_ids).view(ml_dtypes.bfloat16)  # [C]
    cid_b = np.ascontiguousarray(np.broadcast_to(cid_bf[None, :], (128, C)))

    in_maps = []
    for k in range(NCORES):
        psl = slice(k * PS, (k + 1) * PS)
        csl = slice(k * CS, (k + 1) * CS)
        pf = prev_feat[psl]
        cf = cur_feat[csl]
        pf_n = np.ascontiguousarray(pf.reshape(NPJ, 128, D).transpose(1, 0, 2))
        cfs_n = np.ascontiguousarray(cf.reshape(NCJ, 128, D).transpose(1, 0, 2))
        pfT = np.ascontiguousarray(pf.T.reshape(ND, 128, PS).transpose(1, 0, 2))
        cfsT = np.ascontiguousarray(cf.T.reshape(ND, 128, CS).transpose(1, 0, 2))
        pid_enc = (_encode_ids_u16(prev_ids[psl]).astype(np.uint32) << 16).view(
            np.float32
        )
        pid = np.ascontiguousarray(pid_enc.reshape(NPJ, 128).T)
        in_maps.append(dict(pf_n=pf_n, pfT=pfT, cfs_n=cfs_n, cfsT=cfsT,
                            cid_b=cid_b, pid=pid))
    return in_maps


def run(prev_feat, cur_feat, prev_ids, cur_ids, trace=False, mode="silu", **kw):
    nc = _get_nc(mode)
    in_maps = make_in_maps(prev_feat, cur_feat, prev_ids, cur_ids)
    res = run_bass_kernel_spmd(nc, in_maps, core_ids=list(range(NCORES)),
                               trace=trace, **kw)
    partials = np.array(
        [res.results[i]["out"][0, 0] for i in range(NCORES)], dtype=np.float64
    )
    n = float(P) * float(C)
    if mode == "silu":
        loss = partials.sum() / (SILU_B * n) + LN2 + SILU_C0
    else:
        loss = partials.sum() / n
    return np.float32(loss), res


def kernel(prev_feat, cur_feat, prev_ids, cur_ids):
    loss, _ = run(prev_feat, cur_feat, prev_ids, cur_ids, trace=False)
    return np.asarray(loss, dtype=np.float32)
